# revision 9
# baseline (speedup 1.0000x reference)
"""Multi-head linear attention on Trainium2 — 8-core SPMD, batch+head sharded.

Full-tensor contract: kernel(**inputs) takes the complete Q/K/V
[4, 4096, 1024] f32 arrays, internally shards them across 8 NeuronCores
(core c -> batch c//2, heads 8*(c%2) .. 8*(c%2)+8, i.e. a contiguous
512-column slice of the embedding dim), runs one Bass kernel per core,
and reassembles the full [4, 4096, 1024] f32 output.

Per-core math (H=8 local heads, D=64, L=4096):
    phi = sigmoid(0.6053*x - 4.102)
    kv_ext[h] = phi_K[h]^T @ [V[h] | 1]     # [64, 65], f32 PSUM accum
    numden[h] = phi_Q[h] @ kv_ext[h]        # [L, 65]
    out[h]    = numden[h][:, :64] / numden[h][:, 64:65]

Structure: heads are processed in pairs (one K=128 matmul per pair
computes both heads' kv_ext blocks; one K=128 matmul per pair computes
both numden blocks against a block-diagonal kv operand), and the core's
8 heads are split into 2 column groups of 4 heads processed
back-to-back so group 0's phase-Q / division work overlaps group 1's
K/V streaming. Loads issue on the sync HWDGE ring, stores on the
scalar ring (stores can't head-of-line-block loads). V is cast
f32->bf16 and given its ones column on the otherwise idle GpSimd
engine. Matmul inputs are bf16 (PSUM accumulation stays f32).
"""

import numpy as np

B = 4
L = 4096
E = 1024
NH = 8          # heads per core
D = 64
W = D + 1       # head block width incl. ones/den column
EC = NH * D     # 512 embedding columns per core
P = 128
NT = L // P     # 32 row tiles
G = 2           # head groups per core
GC = EC // G    # 256 columns per group (1 KiB DMA rows)
NPAIR = GC // P  # head pairs per group (2)
TBG = 8         # row tiles per DMA batch -> 1 MiB loads
NBG = NT // TBG  # 4 batches per tensor per group
N_CORES = 8

_CACHE = {}


def _build_nc():
    from contextlib import ExitStack

    import concourse.bacc as bacc
    import concourse.bass as bass
    import concourse.mybir as mybir
    import concourse.tile as tile
    from concourse.masks import make_identity

    f32 = mybir.dt.float32
    bf16 = mybir.dt.bfloat16
    SIG = mybir.ActivationFunctionType.Sigmoid

    nc = bacc.Bacc("TRN2", target_bir_lowering=False, debug=False)
    Q = nc.dram_tensor("Q", [L, EC], f32, kind="ExternalInput").ap()
    K = nc.dram_tensor("K", [L, EC], f32, kind="ExternalInput").ap()
    V = nc.dram_tensor("V", [L, EC], f32, kind="ExternalInput").ap()
    O = nc.dram_tensor("O", [L, EC], f32, kind="ExternalOutput").ap()

    with tile.TileContext(nc) as tc, ExitStack() as ctx:
        singles = ctx.enter_context(tc.tile_pool(name="singles", bufs=1))
        ld = ctx.enter_context(tc.tile_pool(name="ld", bufs=3))
        vb = ctx.enter_context(tc.tile_pool(name="vb", bufs=3))
        ph = ctx.enter_context(tc.tile_pool(name="ph", bufs=3))
        qt = ctx.enter_context(tc.tile_pool(name="qt", bufs=48))
        rcp = ctx.enter_context(tc.tile_pool(name="rcp", bufs=8))
        ob = ctx.enter_context(tc.tile_pool(name="ob", bufs=3))
        pt = ctx.enter_context(tc.tile_pool(name="pt", bufs=2, space="PSUM"))
        pn = ctx.enter_context(tc.tile_pool(name="pn", bufs=2, space="PSUM"))
        pk = ctx.enter_context(tc.tile_pool(name="pk", bufs=1, space="PSUM"))

        ident = singles.tile([P, P], f32)
        make_identity(nc, ident)

        sig_bias = singles.tile([P, 1], f32)
        nc.vector.memset(sig_bias, -4.102)

        # Block-diagonal kv operand per head pair: rows 0:64 cols 0:65 hold
        # kv_ext of the even head, rows 64:128 cols 65:130 the odd head.
        kv_bd = singles.tile([P, G * NPAIR, 2 * W], bf16)
        nc.vector.memset(kv_bd, 0.0)

        kv_ps = [pk.tile([P, 2 * W], f32, tag=f"kv{i}", name=f"kv{i}")
                 for i in range(G * NPAIR)]

        for g in range(G):
            cols = slice(g * GC, (g + 1) * GC)

            # ---- K/V streaming: kv_pair += phiK_pair^T @ [V|1]_pair ----
            for ib in range(NBG):
                rows = slice(ib * TBG * P, (ib + 1) * TBG * P)
                k_raw = ld.tile([P, TBG, GC], f32, tag="kraw", name="k_raw")
                nc.sync.dma_start(
                    out=k_raw,
                    in_=K[rows, cols].rearrange("(t p) e -> p t e", p=P),
                )
                v_raw = ld.tile([P, TBG, GC], f32, tag="vraw", name="v_raw")
                nc.sync.dma_start(
                    out=v_raw,
                    in_=V[rows, cols].rearrange("(t p) e -> p t e", p=P),
                )
                phiK = ph.tile([P, TBG, GC], bf16, tag="phiK", name="phiK")
                nc.scalar.activation(
                    out=phiK, in_=k_raw, func=SIG, bias=sig_bias, scale=0.6053
                )
                v_bf = vb.tile([P, TBG, 2 * NPAIR, W], bf16, name="v_bf")
                nc.gpsimd.tensor_copy(
                    out=v_bf.rearrange("p t h w -> p (t h) w")[:, :, 0:D],
                    in_=v_raw.rearrange("p t (h d) -> p (t h) d", d=D),
                )
                nc.gpsimd.memset(
                    v_bf.rearrange("p t h w -> p (t h) w")[:, :, D:W], 1.0)
                for t in range(TBG):
                    for c in range(NPAIR):
                        nc.tensor.matmul(
                            out=kv_ps[g * NPAIR + c],
                            lhsT=phiK[:, t, c * P:(c + 1) * P],
                            rhs=v_bf[:, t, 2 * c:2 * c + 2, :],
                            start=(ib == 0 and t == 0),
                            stop=(ib == NBG - 1 and t == TBG - 1),
                        )
            for c in range(NPAIR):
                pg = g * NPAIR + c
                nc.vector.tensor_copy(
                    out=kv_bd[0:D, pg, 0:W], in_=kv_ps[pg][0:D, 0:W])
                nc.vector.tensor_copy(
                    out=kv_bd[D:P, pg, W:2 * W], in_=kv_ps[pg][D:P, W:2 * W])

            # ---- Q streaming: transpose raw Q on PE, sigmoid PSUM->SBUF
            # on ACT, one matmul per pair, divide on DVE ----
            for ib in range(NBG):
                rows = slice(ib * TBG * P, (ib + 1) * TBG * P)
                q_raw = ld.tile([P, TBG, GC], f32, tag="qraw", name="q_raw")
                nc.sync.dma_start(
                    out=q_raw,
                    in_=Q[rows, cols].rearrange("(t p) e -> p t e", p=P),
                )
                out_t = ob.tile([P, TBG, GC], f32, name="out_t")
                for t in range(TBG):
                    for c in range(NPAIR):
                        tp = pt.tile([P, P], f32, tag="tp", name="tp")
                        nc.tensor.transpose(
                            out=tp, in_=q_raw[:, t, c * P:(c + 1) * P],
                            identity=ident,
                        )
                        qtT = qt.tile([P, P], bf16, tag="qtT", name="qtT")
                        nc.scalar.activation(
                            out=qtT, in_=tp, func=SIG, bias=sig_bias,
                            scale=0.6053,
                        )
                        num = pn.tile([P, 2, W], f32, tag="num", name="num")
                        nc.tensor.matmul(
                            out=num.rearrange("p a b -> p (a b)"),
                            lhsT=qtT,
                            rhs=kv_bd[:, g * NPAIR + c, :],
                        )
                        r = rcp.tile([P, 2], f32, tag="r", name="r")
                        nc.vector.reciprocal(out=r, in_=num[:, :, D])
                        r_bc = bass.AP(
                            tensor=r.tensor, offset=r.offset,
                            ap=[r.ap[0], r.ap[1], [0, D]],
                        )
                        nc.vector.tensor_tensor(
                            out=out_t[:, t, c * P:(c + 1) * P].rearrange(
                                "p (a d) -> p a d", a=2),
                            in0=num[:, :, 0:D],
                            in1=r_bc,
                            op=mybir.AluOpType.mult,
                        )
                nc.scalar.dma_start(
                    out=O[rows, cols].rearrange("(t p) e -> p t e", p=P),
                    in_=out_t,
                )

    nc.compile()
    return nc


def _get_nc():
    if "nc" not in _CACHE:
        _CACHE["nc"] = _build_nc()
    return _CACHE["nc"]


def _shard(arr):
    """Full [B, L, E] f32 -> list of 8 per-core [L, EC] slices."""
    out = []
    for c in range(N_CORES):
        b, g = divmod(c, 2)
        out.append(np.ascontiguousarray(arr[b, :, g * EC:(g + 1) * EC]))
    return out


def run_sharded(in_maps, trace=False, trace_cores=None):
    from concourse.bass_utils import run_bass_kernel_spmd

    nc = _get_nc()
    kwargs = {}
    if trace:
        kwargs = dict(trace=True, trace_cores=trace_cores or [0])
    return run_bass_kernel_spmd(nc, in_maps, core_ids=list(range(N_CORES)), **kwargs)


def kernel(**inputs):
    Q = np.ascontiguousarray(np.asarray(inputs["Q"], dtype=np.float32))
    K = np.ascontiguousarray(np.asarray(inputs["K"], dtype=np.float32))
    V = np.ascontiguousarray(np.asarray(inputs["V"], dtype=np.float32))
    qs, ks, vs = _shard(Q), _shard(K), _shard(V)
    in_maps = [{"Q": qs[c], "K": ks[c], "V": vs[c]} for c in range(N_CORES)]
    res = run_sharded(in_maps)
    out = np.empty((B, L, E), dtype=np.float32)
    for c in range(N_CORES):
        b, g = divmod(c, 2)
        out[b, :, g * EC:(g + 1) * EC] = res.results[c]["O"]
    return out
